# revision 46
# baseline (speedup 1.0000x reference)
"""Trainium2 Bass kernel for NeuronLlama4VisionMLP (fused residual-add +
RMSNorm + up-proj + GELU + down-proj).

Distribution: data-parallel over the 16384 tokens -> 2048 tokens per core,
full weights replicated per core, no collectives.

Host side (cheap elementwise / repack prep):
  - h = x + residual  (this is also the module's second output)
  - per-token rsqrt(mean(h^2)+eps) scale and ln_w are folded into the
    device inputs: normed = h * s, W_up' = ln_w[:,None] * W_up
  - normed is shipped transposed + chunk-repacked fp16 so each chunk is
    one fully HBM-contiguous DMA; the device returns out^T per m-tile and
    b_down is added on host.

Device side per core (T=2048 tokens, H=1408 -> KH=11 tiles, I=5632 ->
KI=44 tiles), processed as 4 passes over 512-token chunks; everything in
fp16 so every matmul runs at the 1 col/cycle @2.4GHz peak (f32r weights
pay ~+11ns/MM in LDWEIGHTS):
    up:   psum[i, c] = sum_k wup[k, i].T @ nt[k, c]      (11-MM chains)
    gelu: act[i, c] = Gelu(psum + b_up[i])               (scalar engine)
    down: psum[m, c] = sum_i wdn[i, m].T @ act[i, c]     (44-MM chains)
    out^T[m, c] -> HBM (f32)

Weights are re-streamed per chunk pass (4x wup, 4x wdn ~ 127MB/core
total) -- the ~360GB/s/core HBM fabric absorbs it, so never the
bottleneck. In exchange act SBUF drops to 5.9MB and chain 0 starts at
~20us (vs 25us baseline) with the HAM clock pre-warmed.

Scheduling learned the hard way (see per-queue notes in build_bass):
the three DMA queues (sync/scalar/gpsimd) share ~360GB/s; weight tiles
move as full-tile dense descriptors; the prologue is a hand-ordered
wavefront with a NWARM junk-matmul burst bridging until data arrives so
the PE clock gate (1.2->2.4GHz) never re-cools; wdn/nt prefetches are
gated to late gelus so they can't dilute the pass-0 wup catch-up
stream; outs ride sync; psum->sbuf copies ride the idle vector engine;
scalar runs the gelus.
"""
import sys

sys.path.insert(0, "/opt/trn_rl_repo")

import numpy as np
import ml_dtypes
import concourse.bass as bass
from concourse import bacc
import concourse.mybir as mybir
from concourse.tile import TileContext
from concourse.bass_utils import run_bass_kernel_spmd

# Problem shape (hardcoded per contract)
B, S, H, I = 16, 1024, 1408, 5632
EPS = 1e-6
NCORES = 8
P = 128
T_CORE = (B * S) // NCORES       # 2048 tokens per core
KH = H // P                      # 11 k-tiles of H
KI = I // P                      # 44 k-tiles of I
IC = 4                           # i-chunks in down weight repack
ISUB = KI // IC                  # 11 i-subtiles per chunk
CH = 512                         # token chunk width (= max fp32-psum MM N)
NCH = T_CORE // CH               # 4 chunk passes per core
NTSPLIT = 6                      # nt chunk DMA k-split
NWARM = 10                       # HAM warmup matmuls

F16 = mybir.dt.float16
F32 = mybir.dt.float32





def build_bass():
    nc = bacc.Bacc(None, target_bir_lowering=False)

    # all weight/act DMAs are HBM-contiguous with multi-KB per-partition runs
    nt = nc.declare_dram_parameter("nt", [NCH, P, KH, CH], F16, isOutput=False)
    wup = nc.declare_dram_parameter("wup", [KI // 2, P, 2, KH, P], F16, isOutput=False)
    wdn = nc.declare_dram_parameter("wdn", [KH, P, IC, ISUB, P], F16, isOutput=False)
    # host-transposed [P, KI] so the DMA is one contiguous run per
    # partition; a "(i p) -> p i" rearrange of the flat vector emits 5632
    # single-element descriptors that clog the issuing queue for ~20us
    bup = nc.declare_dram_parameter("bup", [P, KI], F32, isOutput=False)
    ot = nc.declare_dram_parameter("ot", [KH, P, T_CORE], F32, isOutput=True)

    with TileContext(nc) as tc:
        with (
            tc.tile_pool(name="const", bufs=1) as constp,
            tc.tile_pool(name="ntp", bufs=2) as ntp,
            tc.tile_pool(name="wupp", bufs=6) as wupp,
            tc.tile_pool(name="wdnp", bufs=4) as wdnp,
            tc.tile_pool(name="actp", bufs=KI + 2) as actp,
            tc.tile_pool(name="outp", bufs=4) as outp,
            tc.tile_pool(name="psu", bufs=4, space="PSUM") as psu,
            tc.tile_pool(name="psd", bufs=4, space="PSUM") as psd,
        ):
            # Measured per-queue DMA throughput: gpsimd (SWDGE) ~200GB/s;
            # sync/scalar (HWDGE) only ~70-90GB/s each. Transfers issued on
            # one queue share its bandwidth concurrently, and throughput
            # also degrades below ~2KB per-partition runs. So: steady-state
            # weight tiles move as full-tile single descriptors (5.6-11.3KB
            # runs) weighted 2:1 toward gpsimd, and the prologue hand-
            # schedules the first ~3MB by need-time with the critical bytes
            # leading the gpsimd queue. Non-critical prefetches are gated
            # behind the first gelus (the in-order scalar queue can't reach
            # their dma_starts before the preceding ACTIVATE retires).
            nt_tiles = {}

            def fetch_nt(c):
                # non-critical prefetch, issued gelu-gated on scalar
                t = ntp.tile([P, KH, CH], F16, tag="nt", name=f"nt{c}")
                nc.scalar.dma_start(out=t[:], in_=nt[c])
                nt_tiles[c] = t

            def issue_wup(ip, eng):
                t = wupp.tile([P, 2, KH, P], F16, tag="wup", name=f"wup{ip}")
                eng.dma_start(out=t[:], in_=wup[ip])
                return t

            def issue_wdn(m, eng):
                t = wdnp.tile([P, IC, ISUB, P], F16, tag="wdn", name=f"wdn{m}")
                eng.dma_start(out=t[:], in_=wdn[m])
                return t

            # ---- prologue: full-tile dense descriptors, need-ordered per
            # queue; only chain-0..8-critical bytes at t0. bup leads sync:
            # gelu0 (and the GELU table load) block on it, and a starved
            # bup cascades into a psum-pool stall.
            wup_pre = {}
            nt0 = ntp.tile([P, KH, CH], F16, tag="nt", name="nt0")
            nt_tiles[0] = nt0
            w0 = wupp.tile([P, 2, KH, P], F16, tag="wup", name="wup0")
            w1 = wupp.tile([P, 2, KH, P], F16, tag="wup", name="wup1")
            w4p = wupp.tile([P, 2, KH, P], F16, tag="wup", name="wup4p")
            w2p = wupp.tile([P, 2, KH, P], F16, tag="wup", name="wup2p")
            w5p = wupp.tile([P, 2, KH, P], F16, tag="wup", name="wup5p")
            wup_pre.update({0: w0, 1: w1, 2: w2p, 4: w4p, 5: w5p})
            bup_sb = constp.tile([P, KI], F32)
            nc.gpsimd.dma_start(out=w0[:], in_=wup[0])
            nc.gpsimd.dma_start(out=nt0[:, 0:3], in_=nt[0][:, 0:3])
            nc.gpsimd.dma_start(out=w2p[:], in_=wup[2])
            nc.sync.dma_start(out=bup_sb[:], in_=bup[:, 0:KI])
            nc.sync.dma_start(out=nt0[:, 3:6], in_=nt[0][:, 3:6])
            nc.sync.dma_start(out=w1[:], in_=wup[1])
            nc.gpsimd.dma_start(out=w5p[:], in_=wup[5])
            nc.scalar.dma_start(out=nt0[:, 6:8], in_=nt[0][:, 6:8])
            nc.scalar.dma_start(out=nt0[:, 8:KH], in_=nt[0][:, 8:KH])
            nc.scalar.dma_start(out=w4p[:], in_=wup[4])

            # PE warmup: ~10 junk matmuls flip the HAM clock gate
            # (1.2->2.4GHz) while the prologue DMAs are still in flight,
            # so the first real chains run at full clock.
            wa = constp.tile([P, CH], F16)
            wb = constp.tile([P, P], F16)
            nc.vector.memset(wa[:], 0.0)
            nc.vector.memset(wb[:], 0.0)
            for _ in range(NWARM):
                pw = psu.tile([P, CH], F32, tag="psu", name="pw")
                nc.tensor.matmul(pw[:], wb[:], wa[:], start=True, stop=True)

            for c in range(NCH):
                ntc = nt_tiles.pop(c)

                # ---- up projection + gelu over chunk c ----
                acts = []
                wdn_pre = []
                for ip in range(KI // 2):
                    if ip in wup_pre:
                        wupb = wup_pre.pop(ip)
                    else:
                        wupb = issue_wup(ip, nc.sync if ip % 3 == 0 else nc.gpsimd)
                    for half in range(2):
                        i = 2 * ip + half
                        ps = psu.tile([P, CH], F32, tag="psu")
                        for k in range(KH):
                            nc.tensor.matmul(
                                ps[:],
                                wupb[:, half, k],
                                ntc[:, k],
                                start=(k == 0),
                                stop=(k == KH - 1),
                            )
                        a = actp.tile([P, CH], F16, tag="act", name=f"act{i}")
                        nc.scalar.activation(
                            a[:],
                            ps[:],
                            mybir.ActivationFunctionType.Gelu,
                            bias=bup_sb[:, i : i + 1],
                            scale=1.0,
                        )
                        acts.append(a)
                        # gelu-gated prefetches (see t0 comment above).
                        # Pass 0 additionally gates wup[2]/wup[3] so the t0
                        # burst stays minimal while the stream still leads
                        # consumption by ~2 tiles.
                        # wdn/nt prefetches are needed only at the down
                        # phase (~100us later); gating them at late gelus
                        # keeps the early fabric bandwidth for the wup
                        # catch-up stream.
                        sched = (
                            {1: "wup3", 16: "wdn0", 24: "wdn1", 30: "nt"}
                            if c == 0
                            else {16: "wdn0", 24: "wdn1", 30: "nt"}
                        )
                        ev = sched.get(i)
                        if ev == "wup2":
                            wup_pre[2] = issue_wup(2, nc.scalar)
                        elif ev == "wup3":
                            wup_pre[3] = issue_wup(3, nc.scalar)
                        elif ev == "nt":
                            if c + 1 < NCH:
                                fetch_nt(c + 1)
                        elif ev == "wdn0":
                            wdn_pre.append(issue_wdn(0, nc.scalar))
                        elif ev == "wdn1":
                            wdn_pre.append(issue_wdn(1, nc.scalar))

                # ---- down projection over chunk c ----
                for m in range(KH):
                    if m < len(wdn_pre):
                        wdnb = wdn_pre[m]
                    else:
                        wdnb = issue_wdn(m, nc.scalar if m % 3 == 0 else nc.gpsimd)
                    ps2 = psd.tile([P, CH], F32, tag="psd")
                    for i in range(KI):
                        nc.tensor.matmul(
                            ps2[:],
                            wdnb[:, i // ISUB, i % ISUB],
                            acts[i][:],
                            start=(i == 0),
                            stop=(i == KI - 1),
                        )
                    osb = outp.tile([P, CH], F32, tag="osb")
                    tok = slice(c * CH, (c + 1) * CH)
                    out_eng = nc.sync
                    if c == NCH - 1 and m == KH - 1:
                        # split the final output so copy/DMA pipeline
                        HC = CH // 2
                        nc.vector.tensor_copy(out=osb[:, 0:HC], in_=ps2[:, 0:HC])
                        out_eng.dma_start(
                            out=ot[m][:, c * CH : c * CH + HC], in_=osb[:, 0:HC]
                        )
                        nc.vector.tensor_copy(out=osb[:, HC:CH], in_=ps2[:, HC:CH])
                        out_eng.dma_start(
                            out=ot[m][:, c * CH + HC : (c + 1) * CH],
                            in_=osb[:, HC:CH],
                        )
                    else:
                        nc.vector.tensor_copy(out=osb[:], in_=ps2[:])
                        out_eng.dma_start(out=ot[m][:, tok], in_=osb[:])
    nc.compile()
    return nc


_CACHED = {}


def _get_nc():
    if "nc" not in _CACHED:
        _CACHED["nc"] = build_bass()
    return _CACHED["nc"]


def _prep_host(x, residual, ln_w, W_up, b_up, W_down):
    """Host-side prep: h, chunk-repacked fp16 normed^T per core, weights."""
    h = x + residual                                   # [B,S,H] f32
    hf = h.reshape(-1, H)                              # [16384, H]
    var = np.mean(np.square(hf), axis=-1)              # f32
    s = 1.0 / np.sqrt(var + EPS)                       # f32
    normed = (hf * s[:, None]).astype(np.float16)      # ln_w folded into W

    Wup_p = (W_up * ln_w[:, None]).astype(np.float16)  # [H, I]
    # wup[ip, p, b, k, il] = Wup_p[k*128+p, (2*ip+b)*128+il]
    WUP = np.ascontiguousarray(
        Wup_p.reshape(KH, P, KI // 2, 2, P).transpose(2, 1, 3, 0, 4)
    )                                                  # [KI/2,P,2,KH,P] f16
    # wdn[m, p, ic, isub, cc] = W_down[(ic*ISUB+isub)*128+p, m*128+cc]
    WDN = np.ascontiguousarray(
        W_down.astype(np.float16).reshape(IC, ISUB, P, KH, P).transpose(3, 2, 0, 1, 4)
    )                                                  # [KH,P,IC,ISUB,P] f16

    # bup[p, i] = b_up[i*128+p] -> contiguous [P, KI] DMA
    BUP = np.ascontiguousarray(b_up.astype(np.float32).reshape(KI, P).T)

    in_maps = []
    for c in range(NCORES):
        blk = normed[c * T_CORE : (c + 1) * T_CORE]    # [T_CORE, H] f16
        # nt[ch, p, k, j] = normed^T[k*128+p, ch*512+j]
        ntc = np.ascontiguousarray(
            blk.T.reshape(KH, P, NCH, CH).transpose(2, 1, 0, 3)
        )                                              # [NCH,P,KH,CH] f16
        in_maps.append({"nt": ntc, "wup": WUP, "wdn": WDN, "bup": BUP})
    return h, in_maps


_RESET_DONE = {}


def _maybe_reset_device():
    """Best-effort terminal NRT reset so a previously wedged device can't
    hang the run. No-op when the axon .so or symbol is unavailable."""
    if _RESET_DONE:
        return
    _RESET_DONE["done"] = True
    try:
        import ctypes
        import jax

        jax.devices()
        lib = ctypes.CDLL("/opt/axon/libaxon_pjrt.so")
        if hasattr(lib, "axon_reset"):
            lib.axon_reset.restype = ctypes.c_int64
            lib.axon_reset()
    except Exception:
        pass


def _run(in_maps, **kw):
    _maybe_reset_device()
    nc = _get_nc()
    return run_bass_kernel_spmd(nc, in_maps, core_ids=list(range(NCORES)), **kw)


def _assemble(results, b_down):
    # ot[m, p, t] -> out[t, m*128+p]
    outs = [r["ot"].transpose(2, 0, 1).reshape(T_CORE, H) for r in results]
    out = np.concatenate(outs, axis=0).reshape(B, S, H)
    out = out + b_down.astype(np.float32)
    return out


def kernel(x, residual, ln_w, W_up, b_up, W_down, b_down):
    x = np.asarray(x, dtype=np.float32)
    residual = np.asarray(residual, dtype=np.float32)
    ln_w = np.asarray(ln_w, dtype=np.float32)
    W_up = np.asarray(W_up, dtype=np.float32)
    b_up = np.asarray(b_up, dtype=np.float32)
    W_down = np.asarray(W_down, dtype=np.float32)
    b_down = np.asarray(b_down, dtype=np.float32)

    h, in_maps = _prep_host(x, residual, ln_w, W_up, b_up, W_down)
    res = _run(in_maps)
    out = _assemble(res.results, b_down)
    return out, h


def kernel_traced(x, residual, ln_w, W_up, b_up, W_down, b_down, **kw):
    """Like kernel() but with NTFF tracing; returns ((out, h), results)."""
    h, in_maps = _prep_host(
        np.asarray(x, np.float32),
        np.asarray(residual, np.float32),
        np.asarray(ln_w, np.float32),
        np.asarray(W_up, np.float32),
        np.asarray(b_up, np.float32),
        np.asarray(W_down, np.float32),
    )
    res = _run(in_maps, trace=True, **kw)
    out = _assemble(res.results, np.asarray(b_down, np.float32))
    return (out, h), res
